# revision 2
# baseline (speedup 1.0000x reference)
"""Trainium2 Bass kernel for nn_AutoTransformer_27230092656858 (moe_routing).

Math (per the reference):
  h_k    = relu(x @ W1[k] + b1[k])                      for k in 0..3
  flat   = concat_k( where(readout_x==k, h_k @ W2_k + b2_k, 0) )
  out[readout_t - min_t, b] = flat                      (collision-free scatter)

Strategy: token-parallel MoE routing with ALL routing + layout done host-side
at shard time.  The host sorts tokens by readout type, splits each type's
token list evenly across the 8 cores (exact capacities, ~1 token padding),
and hands every core pre-gathered pre-transposed bf16 activations and weights
pre-packed in the exact SBUF image layout, so each DMA is a handful of
multi-KB fully-contiguous per-partition descriptors.

Device kernel per core: dense GEMM pipeline over the 4 heads,
  L1: ht[co, t] = relu(W1k^T-chunks @ xt)          (tokens moving, N<=512)
  L2: out[d, t] = W2k-chunks^T @ ht                (tokens moving, N<=512)
L2 keeps tokens as the *moving* dim so its cost scales with cap rather than
128*ceil(cap/128); output is [D, cap] per head, transposed on the host.

DMA issue is spread across otherwise-idle engine queues (xt/W2/stores on
GpSimd SWDGE, W1 halves on Sync+Vector HWDGE) because descriptor generation
-- not HBM bandwidth -- is the measured bottleneck, and next-head loads are
emitted before prev-head stores so store waits never block loads.
"""

import sys

if "/opt/trn_rl_repo" not in sys.path:
    sys.path.insert(0, "/opt/trn_rl_repo")

import numpy as np

import concourse.bass as bass
import concourse.mybir as mybir
import concourse.tile as tile
from concourse import bacc
from concourse.bass_utils import run_bass_kernel_spmd

# Problem shapes (hardcoded per spec)
S, B, C = 512, 32, 1024
HEAD_DIMS = (2048, 2048, 1024, 512)
K = 4
A = sum(HEAD_DIMS)  # 5632
NCORES = 8
NTOK = S * B  # 16384 total tokens
BASES = [sum(HEAD_DIMS[:k]) for k in range(K)]
BLKW = [min(1024, d) for d in HEAD_DIMS]
NBLK = [-(-d // 1024) for d in HEAD_DIMS]

F32 = mybir.dt.float32
BF16 = mybir.dt.bfloat16
RELU = mybir.ActivationFunctionType.Relu

DEFAULT_CFG = dict(
    w1_bufs=2,
    w2_bufs=6,
    xt_bufs=3,
    ht_bufs=2,
    so_bufs=2,
    l1_psum_bufs=3,
    l2_psum_bufs=4,
    out_dtype="bf16",  # "f32" or "bf16" (host upcasts)
    store_dblks=4,     # d-128-blocks per output store DMA
    first_fast=True,   # fine-grained L1 chunks on head 0 for a short ramp
)

_program_cache: dict = {}


def _chunks_of(n, step):
    """Balanced chunk sizes <= step summing to n."""
    if n <= 0:
        return []
    nch = -(-n // step)
    base = n // nch
    rem = n - base * nch
    return [base + 1] * rem + [base] * (nch - rem)


def _build_program(caps, use_b1, cfg=None):
    """Build + compile the (shared, SPMD) Bass program.

    caps[k]: per-core token capacity for head k (0 = head unused).
    """
    cfg = {**DEFAULT_CFG, **(cfg or {})}
    ODT = F32 if cfg["out_dtype"] == "f32" else BF16
    nc = bacc.Bacc("TRN2", target_bir_lowering=False, debug=False)

    live = [k for k in range(K) if caps[k] > 0]
    xts = {
        k: nc.dram_tensor(f"xt_{k}", [128, 8, caps[k]], BF16, kind="ExternalInput")
        for k in live
    }
    # weights pre-packed on host into SBUF image layouts
    w1 = nc.dram_tensor("w1", [K, 128, 8, C], BF16, kind="ExternalInput")
    if use_b1:
        b1 = nc.dram_tensor("b1", [K, 128, 8], F32, kind="ExternalInput")
    w2 = {
        k: nc.dram_tensor(f"w2_{k}", [128, NBLK[k], 8, BLKW[k]], BF16,
                          kind="ExternalInput")
        for k in live
    }
    # output per head: [D, cap] (token columns; host transposes)
    outs = {
        k: nc.dram_tensor(f"out_{k}", [HEAD_DIMS[k], caps[k]], ODT,
                          kind="ExternalOutput")
        for k in live
    }

    with tile.TileContext(nc) as tc:
        with (
            tc.tile_pool(name="xtp", bufs=cfg["xt_bufs"]) as xtpool,
            tc.tile_pool(name="w1p", bufs=cfg["w1_bufs"]) as w1pool,
            tc.tile_pool(name="w2p", bufs=cfg["w2_bufs"]) as w2pool,
            tc.tile_pool(name="htp", bufs=cfg["ht_bufs"]) as htpool,
            tc.tile_pool(name="sop", bufs=cfg["so_bufs"]) as sopool,
            tc.tile_pool(name="bp", bufs=1) as bpool,
            tc.tile_pool(name="l1ps", bufs=cfg["l1_psum_bufs"], space="PSUM")
                as l1psum,
            tc.tile_pool(name="l2ps", bufs=cfg["l2_psum_bufs"], space="PSUM")
                as l2psum,
        ):
            def l1_sizes(k, first):
                if first and caps[k] > 256:
                    return [128] + _chunks_of(caps[k] - 128, 512)
                return _chunks_of(caps[k], 512)

            def emit_loads(k, first=False):
                capk = caps[k]
                # xt on gpsimd (SWDGE): 1-2 instrs, 8KB runs
                xt = xtpool.tile([128, 8, capk], BF16, tag="xt")
                if first:
                    bnds = np.cumsum([0] + l1_sizes(k, first)).tolist()
                else:
                    bnds = [0, capk]
                for t0, te in zip(bnds[:-1], bnds[1:]):
                    nc.gpsimd.dma_start(xt[:, :, t0:te], xts[k][:, :, t0:te])
                # w1 ci-halves on sync + scalar (HWDGE, 128 x 8KB descs each)
                w1t = w1pool.tile([128, 8, C], BF16, tag="w1")
                nc.sync.dma_start(w1t[:, 0:4, :], w1[k][:, 0:4, :])
                nc.scalar.dma_start(w1t[:, 4:8, :], w1[k][:, 4:8, :])
                # all w2 blocks of this head on gpsimd
                w2cs = []
                for b in range(NBLK[k]):
                    w2c = w2pool.tile([128, 8, BLKW[k]], BF16, tag="w2")
                    nc.gpsimd.dma_start(w2c[:], w2[k][:, b])
                    w2cs.append(w2c)
                b1t = None
                if use_b1:
                    b1t = bpool.tile([128, 8], F32, tag="b1")
                    nc.gpsimd.dma_start(b1t[:], b1[k])
                return xt, w1t, w2cs, b1t

            def layer1(k, w1t, xt, b1t, first=False):
                capk = caps[k]
                ht = htpool.tile([128, 8, capk], BF16, tag="ht")
                n0 = 0
                for nt in l1_sizes(k, first):
                    for m in range(8):
                        ps = l1psum.tile([128, 512], F32, tag="l1")
                        for ci in range(8):
                            nc.tensor.matmul(
                                ps[:, :nt],
                                w1t[:, ci, m * 128 : (m + 1) * 128],
                                xt[:, ci, n0 : n0 + nt],
                                start=(ci == 0),
                                stop=(ci == 7),
                            )
                        if use_b1:
                            nc.scalar.activation(
                                ht[:, m, n0 : n0 + nt], ps[:, :nt], RELU,
                                bias=b1t[:, m : m + 1],
                            )
                        else:
                            nc.scalar.activation(
                                ht[:, m, n0 : n0 + nt], ps[:, :nt], RELU
                            )
                    n0 += nt
                return ht

            def layer2(k, ht, w2cs):
                capk = caps[k]
                D = HEAD_DIMS[k]
                nd = D // 128
                outv = outs[k].rearrange("(b p) t -> p b t", p=128)
                so = sopool.tile([128, nd, capk], ODT, tag="so")
                tchunks = _chunks_of(capk, 512)
                sg = cfg["store_dblks"]
                g0 = 0  # first un-stored d-block
                for db in range(nd):
                    w2c = w2cs[db // 8]
                    dl = (db % 8) * 128
                    n0 = 0
                    for nt in tchunks:
                        ps2 = l2psum.tile([128, 512], F32, tag="l2")
                        for m in range(8):
                            nc.tensor.matmul(
                                ps2[:, :nt],
                                w2c[:, m, dl : dl + 128],
                                ht[:, m, n0 : n0 + nt],
                                start=(m == 0),
                                stop=(m == 7),
                            )
                        nc.vector.tensor_copy(
                            so[:, db, n0 : n0 + nt], ps2[:, :nt]
                        )
                        n0 += nt
                    # staggered stores on gpsimd: flush every sg d-blocks
                    if db + 1 - g0 >= sg or db + 1 == nd:
                        nc.gpsimd.dma_start(
                            outv[:, g0 : db + 1, :], so[:, g0 : db + 1, :]
                        )
                        g0 = db + 1
                return

            # software pipeline, loads one head ahead, stores one head behind:
            # PE order: L1(0) L1(1) L2(0) L1(2) L2(1) L1(3) L2(2) L2(3)
            loads = {live[0]: emit_loads(live[0], first=cfg["first_fast"])}
            l2q = {}
            for i, k in enumerate(live):
                xt, w1t, w2cs, b1t = loads.pop(k)
                ht = layer1(k, w1t, xt, b1t, first=(i == 0 and cfg["first_fast"]))
                if i + 1 < len(live):
                    loads[live[i + 1]] = emit_loads(live[i + 1])
                l2q[k] = (ht, w2cs)
                if i > 0:
                    kp = live[i - 1]
                    hp, wp = l2q.pop(kp)
                    layer2(kp, hp, wp)
            kl = live[-1]
            hp, wp = l2q.pop(kl)
            layer2(kl, hp, wp)

    nc.compile()
    return nc


def _prepare(inputs, cfg=None):
    """Host-side routing + SBUF-image packing for all 8 cores."""
    import ml_dtypes

    x = np.ascontiguousarray(np.asarray(inputs["x"], dtype=np.float32))
    rx = np.asarray(inputs["readout_x"], dtype=np.int64)
    rt = np.asarray(inputs["readout_t"], dtype=np.int64)
    W1 = np.asarray(inputs["W1"], dtype=np.float32)
    b1 = np.asarray(inputs["b1"], dtype=np.float32)
    W2 = [np.asarray(inputs[f"W2_{k}"], dtype=np.float32) for k in range(K)]
    b2 = [np.asarray(inputs[f"b2_{k}"], dtype=np.float32) for k in range(K)]

    x_flat = x.reshape(NTOK, C)
    rx_flat = rx.reshape(-1)
    # output row for each flat token: (rt - min_t)*B + b
    min_t = rt.min(axis=0)
    targ = ((rt - min_t[None, :]) * B
            + np.arange(B, dtype=np.int64)[None, :]).reshape(-1)

    # sort tokens by head, split each head's list evenly over cores
    lists = [np.nonzero(rx_flat == k)[0] for k in range(K)]
    counts = [len(l) for l in lists]
    caps = tuple(-(-c // NCORES) if c else 0 for c in counts)
    use_b1 = bool(np.any(b1))

    key = (caps, use_b1, tuple(sorted((cfg or {}).items())))
    if key not in _program_cache:
        _program_cache[key] = _build_program(caps, use_b1, cfg)
    nc = _program_cache[key]

    # weights in SBUF image layout: [p, sub, cols]
    W1p = np.ascontiguousarray(
        W1.astype(ml_dtypes.bfloat16)
        .reshape(K, 8, 128, C).transpose(0, 2, 1, 3)
    )  # [K, 128, 8, C]
    b1p = np.ascontiguousarray(
        b1.reshape(K, 8, 128).transpose(0, 2, 1)
    )  # [K, 128, 8]
    W2p = []
    for k in range(K):
        wb = W2[k].astype(ml_dtypes.bfloat16).reshape(8, 128, HEAD_DIMS[k])
        # [m, p, d] -> [p, blk, m, wt]
        wb = wb.reshape(8, 128, NBLK[k], BLKW[k]).transpose(1, 2, 0, 3)
        W2p.append(np.ascontiguousarray(wb))

    in_maps = []
    core_ids_per = []  # per core: {k: ids}
    for c in range(NCORES):
        m = {"w1": W1p}
        if use_b1:
            m["b1"] = b1p
        ids_c = {}
        for k in range(K):
            if caps[k] == 0:
                continue
            cnt = counts[k]
            q, r = divmod(cnt, NCORES)
            st = c * q + min(c, r)
            sz = q + (1 if c < r else 0)
            ids = lists[k][st : st + sz]
            ids_c[k] = ids
            idp = np.zeros(caps[k], dtype=np.int64)
            idp[: len(ids)] = ids
            xg = x_flat[idp].astype(ml_dtypes.bfloat16)  # [cap, C]
            m[f"xt_{k}"] = np.ascontiguousarray(
                xg.reshape(caps[k], 8, 128).transpose(2, 1, 0)
            )
            m[f"w2_{k}"] = W2p[k]
        core_ids_per.append(ids_c)
        in_maps.append(m)
    return nc, in_maps, core_ids_per, targ, b2, caps


def _run(inputs, cfg=None, **run_kwargs):
    nc, in_maps, core_ids_per, targ, b2, caps = _prepare(inputs, cfg)
    res = run_bass_kernel_spmd(
        nc, in_maps, core_ids=list(range(NCORES)), **run_kwargs
    )
    out_full = np.zeros((NTOK, A), dtype=np.float32)
    for c in range(NCORES):
        for k, ids in core_ids_per[c].items():
            if len(ids) == 0:
                continue
            blk = np.asarray(res.results[c][f"out_{k}"][:, : len(ids)],
                             dtype=np.float32).T  # [len, D]
            if np.any(b2[k]):
                blk = blk + b2[k][None, :]
            out_full[targ[ids], BASES[k] : BASES[k] + HEAD_DIMS[k]] = blk
    return out_full.reshape(S, B, A), res


def kernel(**inputs) -> np.ndarray:
    full, _ = _run(inputs)
    return full
